# revision 32
# baseline (speedup 1.0000x reference)
# Binary (sign) matmul: out[b,m,n] = sum_k sign(x[b,m,k]) * sign(y[b,n,k]) * x_clip * y_clip
# B=2, M=N=K=4096, fp32 in/out.
#
# Sharding: 8 cores = batch(2) x 2x2 grid over (M, N). Each core computes a
# [2048, 2048] output block from x[b, mh*2048:, :] and y[b, nh*2048:, :].
# The host binds each core's shards in k-major (transposed) layout — pure
# input marshalling; all arithmetic (sign, matmul, clip scaling) runs on
# device.
#
# Per-core device pipeline:
#   DMA fp32 k-major chunks -> ScalarE Sign (fp32 -> fp8e4 +-1, one op per
#   chunk so downstream LDWEIGHTS dedupe keeps working) -> TensorE DoubleRow
#   fp8 matmuls (exact: sums of +-1 in fp32 PSUM) -> DVE spills/close ->
#   DMA out.
#
# Schedule: K split in three phases [4,4,8] DR-steps (kd = 256 k-values).
#   q0 (kd 0-3):  per i-tile, accumulate in PSUM, spill CAST -> fp16 ACC
#                 (|sum| <= 1024, exact in fp16)
#   q1 (kd 4-7):  accumulate, spill ADD into ACC (|sum| <= 2048, exact)
#   H  (kd 8-15): accumulate 8 steps in PSUM, close: fp32 add of PSUM+ACC,
#                 scale by clip product, DMA out.
# All 16 i-tiles march through each phase in sequence; phase boundaries
# match the DMA stream order (k-ascending), so the PE chases the stream
# with bounded stalls instead of serializing behind a full-K dependency.
#
# SBUF: the fp16 ACC is 64KB/partition, so the sign tensors cannot be fully
# resident. They are held as per-phase slabs (8 k-chunks = 16KB each) in
# 3-buffer pools: a phase's slab is dead once its i-march completes, and the
# pool rotation lets the H slabs overwrite the q0 slabs mid-stream.
#
# HAM warmth: PE-idle gaps > ~3.4us re-throttle the PE clock to 1.2 GHz.
# During the two chase windows (q0-i0 and H-i0) the PE waits on chunk
# landings; tiny normal-mode fp8 matmuls that READ freshly-signed slab
# slices fire as the Sign ops complete (~3us cadence), keeping the
# activity monitor busy through the gaps.
import numpy as np

B = 2
M = N = K = 4096
P = 128
MSH, NSH = 2048, 2048      # per-core shard of M, N
KO = K // P                # 32 k-chunks of 128
MT = MSH // P              # 16 m row-tiles
FD = 512                   # matmul free dim
NCH = NSH // FD            # 4 n chunks
NCORES = 8

KD = KO // 2               # 16 DoubleRow k-steps of 256
SLAB = 8                   # k-chunks per slab (4 DR steps)
NSLAB = KO // SLAB         # 4 slabs: q0, q1, Ha, Hb


def _build_program():
    import concourse.bacc as bacc
    import concourse.mybir as mybir
    import concourse.tile as tile
    from concourse.bass import ts

    f32 = mybir.dt.float32
    f16 = mybir.dt.float16
    fp8 = mybir.dt.float8e4
    Sign = mybir.ActivationFunctionType.Sign

    nc = bacc.Bacc(
        "TRN2",
        target_bir_lowering=False,
        debug=False,
        num_devices=NCORES,
    )
    xsT = nc.dram_tensor("xsT", [K, MSH], f32, kind="ExternalInput").ap()
    ysT = nc.dram_tensor("ysT", [K, NSH], f32, kind="ExternalInput").ap()
    clips = nc.dram_tensor("clips", [P, 2], f32, kind="ExternalInput").ap()
    out = nc.dram_tensor("out", [MSH, NSH], f32, kind="ExternalOutput").ap()

    with tile.TileContext(nc) as tc:
        with (
            tc.tile_pool(name="constp", bufs=1) as constp,
            tc.tile_pool(name="sxp", bufs=3) as sxp,
            tc.tile_pool(name="syp", bufs=3) as syp,
            tc.tile_pool(name="accp", bufs=1) as accp,
            tc.tile_pool(name="stagep", bufs=5) as stagep,
            tc.tile_pool(name="outp", bufs=3) as outp,
            tc.tile_pool(name="psump", bufs=8, space="PSUM") as psump,
        ):
            # clip product, replicated per-partition: [P, 1]
            clip_sb = constp.tile([P, 2], f32)
            nc.sync.dma_start(clip_sb[:], clips)
            clip_prod = constp.tile([P, 1], f32)
            nc.vector.tensor_tensor(
                clip_prod[:], clip_sb[:, 0:1], clip_sb[:, 1:2],
                mybir.AluOpType.mult,
            )

            # fp16 accumulator for the two quarter-K spills (exact <= 2048)
            ACC = accp.tile([P, MT, NCH, FD], f16, name="ACC")

            def prep(src_dram, ko, dst, kol):
                st = stagep.tile([P, MSH], f32, name="st", tag="stage")
                nc.sync.dma_start(st[:], src_dram[ts(ko, P), :])
                nc.scalar.activation(dst[:, kol, :], st[:], Sign)

            # Sign slabs, filled in stream order (x and y interleaved per ko).
            # Slab s covers k-chunks [s*SLAB, (s+1)*SLAB). With 3-buffer
            # pools, slab 3 reuses slab 0's memory; its preps are emitted
            # after the q0 march (the last reader of slab 0) so the pool's
            # WAR tracking sees the reads first in program order.
            xslabs, yslabs = [], []

            def emit_slab(s):
                sx = sxp.tile([P, SLAB, MSH], fp8, name=f"sx{s}", tag="sx")
                sy = syp.tile([P, SLAB, NSH], fp8, name=f"sy{s}", tag="sy")
                xslabs.append(sx)
                yslabs.append(sy)
                for kol in range(SLAB):
                    ko = s * SLAB + kol
                    prep(xsT, ko, sx, kol)
                    prep(ysT, ko, sy, kol)

            for s in range(3):
                emit_slab(s)

            def group_mm(pss, i, s, kdl, first, last):
                for nch in range(NCH):
                    nc.tensor.matmul(
                        pss[nch][:],
                        lhsT=xslabs[s][:, 2 * kdl : 2 * kdl + 2, ts(i, P)],
                        rhs=yslabs[s][:, 2 * kdl : 2 * kdl + 2, ts(nch, FD)],
                        start=first,
                        stop=last,
                        perf_mode=mybir.MatmulPerfMode.DoubleRow,
                    )

            def mm_group(i, kds):
                """One i-tile's accumulation over DR k-steps into 4 banks."""
                pss = [
                    psump.tile([P, FD], f32, name=f"ps{n}", tag="ps")
                    for n in range(NCH)
                ]
                last = len(kds) - 1
                for j, (s, kdl) in enumerate(kds):
                    group_mm(pss, i, s, kdl, j == 0, j == last)
                return pss

            def pair_chase(kds):
                """i0 and i1 march kd-major (8 banks) right behind the DMA
                stream during a chase window; returns their psum banks."""
                pair = [
                    [
                        psump.tile([P, FD], f32, name=f"ps{n}", tag="ps")
                        for n in range(NCH)
                    ]
                    for _ in range(2)
                ]
                last = len(kds) - 1
                for j, (s, kdl) in enumerate(kds):
                    for i in (0, 1):
                        group_mm(pair[i], i, s, kdl, j == 0, j == last)
                return pair

            Q0 = [(0, kdl) for kdl in range(4)]
            Q1 = [(1, kdl) for kdl in range(4)]
            HH = [(2, kdl) for kdl in range(4)] + [(3, kdl) for kdl in range(4)]

            # ---- q0: kd 0..3, spill CAST -> ACC ----
            # i0+i1 chase the stream kd-major (all 8 banks), then i2..15
            # march on resident data.
            pair = pair_chase(Q0)
            for i in (0, 1):
                for nch in range(NCH):
                    nc.vector.tensor_copy(
                        out=ACC[:, i, nch, :], in_=pair[i][nch][:]
                    )
            for i in range(2, MT):
                pss = mm_group(i, Q0)
                for nch in range(NCH):
                    nc.vector.tensor_copy(
                        out=ACC[:, i, nch, :], in_=pss[nch][:]
                    )

            # slab 3 (ko 24-31) reuses slab 0's buffers — emit after q0.
            emit_slab(3)

            # ---- q1: kd 4..7, spill ADD -> ACC ----
            for i in range(MT):
                pss = mm_group(i, Q1)
                for nch in range(NCH):
                    nc.vector.tensor_tensor(
                        ACC[:, i, nch, :], pss[nch][:], ACC[:, i, nch, :],
                        mybir.AluOpType.add,
                    )

            # ---- H: kd 8..15, close: PSUM + ACC, scale, out ----
            def close(i, pss):
                for nch in range(NCH):
                    ot = outp.tile([P, FD], f32, name="ot")
                    nc.vector.tensor_tensor(
                        ot[:], pss[nch][:], ACC[:, i, nch, :],
                        mybir.AluOpType.add,
                    )
                    nc.vector.tensor_scalar_mul(ot[:], ot[:], clip_prod[:])
                    nc.sync.dma_start(out[ts(i, P), ts(nch, FD)], ot[:])

            # i0+i1 chase the stream tail together, then i2..15 march.
            pair = pair_chase(HH)
            for i in (0, 1):
                close(i, pair[i])
            for i in range(2, MT):
                close(i, mm_group(i, HH))

    nc.compile()
    _dedupe_ldweights(nc)
    return nc


def _dedupe_ldweights(nc):
    """Drop redundant standalone InstLdweights left by bacc's matmul split.

    Consecutive matmuls sharing one stationary tile still get one
    InstLdweights each; an InstLdweights identical to the previous one
    (same AP, same mode) with no semaphore waits/updates is a no-op."""
    removed = 0
    for blk in nc.m.functions[0].blocks:
        prev_key = None
        keep = []
        for inst in blk.instructions:
            nm = type(inst).__name__
            if nm == "InstLdweights":
                pap = inst.ins[0]
                key = (
                    pap.memref,
                    pap.offset,
                    str(pap.ap),
                    str(pap.dtype),
                    str(inst.perf_mode),
                    str(inst.is_transpose),
                )
                if (
                    key == prev_key
                    and not inst.has_wait()
                    and not inst.has_update()
                ):
                    removed += 1
                    continue
                prev_key = key
            keep.append(inst)
        if removed:
            blk.instructions = keep
    return removed


_PROGRAM_CACHE = None


def _get_program():
    global _PROGRAM_CACHE
    if _PROGRAM_CACHE is None:
        _PROGRAM_CACHE = _build_program()
    return _PROGRAM_CACHE


def _shard_inputs(x, y, x_clip, y_clip):
    x = np.asarray(x, dtype=np.float32)
    y = np.asarray(y, dtype=np.float32)
    clips = np.empty((P, 2), dtype=np.float32)
    clips[:, 0] = np.float32(x_clip)
    clips[:, 1] = np.float32(y_clip)
    in_maps = []
    for c in range(NCORES):
        b, mh, nh = c // 4, (c % 4) // 2, c % 2
        in_maps.append(
            {
                "xsT": np.ascontiguousarray(x[b, mh * MSH : (mh + 1) * MSH, :].T),
                "ysT": np.ascontiguousarray(y[b, nh * NSH : (nh + 1) * NSH, :].T),
                "clips": clips,
            }
        )
    return in_maps


def run_sharded(x, y, x_clip, y_clip, trace=False, **kwargs):
    """Run the SPMD kernel; returns (out, BassKernelResults)."""
    from concourse.bass_utils import run_bass_kernel_spmd

    nc = _get_program()
    in_maps = _shard_inputs(x, y, x_clip, y_clip)
    res = run_bass_kernel_spmd(
        nc, in_maps, core_ids=list(range(NCORES)), trace=trace, **kwargs
    )
    out = np.empty((B, M, N), dtype=np.float32)
    for c in range(NCORES):
        b, mh, nh = c // 4, (c % 4) // 2, c % 2
        out[b, mh * MSH : (mh + 1) * MSH, nh * NSH : (nh + 1) * NSH] = res.results[
            c
        ]["out"]
    return out, res


def kernel(x, y, x_clip, y_clip):
    out, _ = run_sharded(x, y, x_clip, y_clip, trace=False)
    return out


# revision 33
# speedup vs baseline: 1.0895x; 1.0895x over previous
# Binary (sign) matmul: out[b,m,n] = sum_k sign(x[b,m,k]) * sign(y[b,n,k]) * x_clip * y_clip
# B=2, M=N=K=4096, fp32 in/out.
#
# Sharding: 8 cores = batch(2) x 2x2 grid over (M, N). Each core computes a
# [2048, 2048] output block from x[b, mh*2048:, :] and y[b, nh*2048:, :].
# The host binds each core's shards in k-major (transposed) layout — pure
# input marshalling; all arithmetic (sign, matmul, clip scaling) runs on
# device.
#
# Per-core device pipeline:
#   DMA fp32 k-major chunks -> ScalarE Sign (fp32 -> fp8e4 +-1, one op per
#   chunk so downstream LDWEIGHTS dedupe keeps working) -> TensorE DoubleRow
#   fp8 matmuls (exact: sums of +-1 in fp32 PSUM) -> DVE spills/close ->
#   DMA out.
#
# Schedule: K split in three phases [4,4,8] DR-steps (kd = 256 k-values).
#   q0 (kd 0-3):  per i-tile, accumulate in PSUM, spill CAST -> fp16 ACC
#                 (|sum| <= 1024, exact in fp16)
#   q1 (kd 4-7):  accumulate, spill ADD into ACC (|sum| <= 2048, exact)
#   H  (kd 8-15): accumulate 8 steps in PSUM, close: fp32 add of PSUM+ACC,
#                 scale by clip product, DMA out.
# All 16 i-tiles march through each phase in sequence; phase boundaries
# match the DMA stream order (k-ascending), so the PE chases the stream
# with bounded stalls instead of serializing behind a full-K dependency.
#
# SBUF: the fp16 ACC is 64KB/partition, so the sign tensors cannot be fully
# resident. They are held as per-phase slabs (8 k-chunks = 16KB each) in
# 3-buffer pools: a phase's slab is dead once its i-march completes, and the
# pool rotation lets the H slabs overwrite the q0 slabs mid-stream.
#
# HAM warmth: PE-idle gaps > ~3.4us re-throttle the PE clock to 1.2 GHz.
# During the two chase windows (q0-i0 and H-i0) the PE waits on chunk
# landings; tiny normal-mode fp8 matmuls that READ freshly-signed slab
# slices fire as the Sign ops complete (~3us cadence), keeping the
# activity monitor busy through the gaps.
import numpy as np

B = 2
M = N = K = 4096
P = 128
MSH, NSH = 2048, 2048      # per-core shard of M, N
KO = K // P                # 32 k-chunks of 128
MT = MSH // P              # 16 m row-tiles
FD = 512                   # matmul free dim
NCH = NSH // FD            # 4 n chunks
NCORES = 8

KD = KO // 2               # 16 DoubleRow k-steps of 256
SLAB = 8                   # k-chunks per slab (4 DR steps)
NSLAB = KO // SLAB         # 4 slabs: q0, q1, Ha, Hb


def _build_program():
    import concourse.bacc as bacc
    import concourse.mybir as mybir
    import concourse.tile as tile
    from concourse.bass import ts

    f32 = mybir.dt.float32
    f16 = mybir.dt.float16
    fp8 = mybir.dt.float8e4
    Sign = mybir.ActivationFunctionType.Sign

    nc = bacc.Bacc(
        "TRN2",
        target_bir_lowering=False,
        debug=False,
        num_devices=NCORES,
    )
    xsT = nc.dram_tensor("xsT", [K, MSH], f32, kind="ExternalInput").ap()
    ysT = nc.dram_tensor("ysT", [K, NSH], f32, kind="ExternalInput").ap()
    clips = nc.dram_tensor("clips", [P, 2], f32, kind="ExternalInput").ap()
    out = nc.dram_tensor("out", [MSH, NSH], f32, kind="ExternalOutput").ap()

    with tile.TileContext(nc) as tc:
        with (
            tc.tile_pool(name="constp", bufs=1) as constp,
            tc.tile_pool(name="sxp", bufs=3) as sxp,
            tc.tile_pool(name="syp", bufs=3) as syp,
            tc.tile_pool(name="accp", bufs=1) as accp,
            tc.tile_pool(name="stagep", bufs=5) as stagep,
            tc.tile_pool(name="outp", bufs=3) as outp,
            tc.tile_pool(name="psump", bufs=7, space="PSUM") as psump,
            tc.tile_pool(name="dpsump", bufs=1, space="PSUM") as dpsump,
        ):
            # clip product, replicated per-partition: [P, 1]
            clip_sb = constp.tile([P, 2], f32)
            nc.sync.dma_start(clip_sb[:], clips)
            clip_prod = constp.tile([P, 1], f32)
            nc.vector.tensor_tensor(
                clip_prod[:], clip_sb[:, 0:1], clip_sb[:, 1:2],
                mybir.AluOpType.mult,
            )

            # fp16 accumulator for the two quarter-K spills (exact <= 2048)
            ACC = accp.tile([P, MT, NCH, FD], f16, name="ACC")

            def prep(src_dram, ko, dst, kol):
                st = stagep.tile([P, MSH], f32, name="st", tag="stage")
                nc.sync.dma_start(st[:], src_dram[ts(ko, P), :])
                nc.scalar.activation(dst[:, kol, :], st[:], Sign)

            dwarm = constp.tile([P, 2], fp8)
            nc.vector.memset(dwarm[:], 0)

            def warm_mm(src, kol):
                dps = dpsump.tile([2, 32], f32, name="dps", tag="dps")
                nc.tensor.matmul(
                    dps[:], lhsT=dwarm[:, :2], rhs=src[:, kol, 0:32],
                    start=True, stop=True,
                )

            # Sign slabs, filled in stream order (x and y interleaved per ko).
            # Slab s covers k-chunks [s*SLAB, (s+1)*SLAB). With 3-buffer
            # pools, slab 3 reuses slab 0's memory; its preps are emitted
            # after the q0 march (the last reader of slab 0) so the pool's
            # WAR tracking sees the reads first in program order.
            xslabs, yslabs = [], []

            def emit_slab(s):
                sx = sxp.tile([P, SLAB, MSH], fp8, name=f"sx{s}", tag="sx")
                sy = syp.tile([P, SLAB, NSH], fp8, name=f"sy{s}", tag="sy")
                xslabs.append(sx)
                yslabs.append(sy)
                for kol in range(SLAB):
                    ko = s * SLAB + kol
                    prep(xsT, ko, sx, kol)
                    prep(ysT, ko, sy, kol)

            for s in range(3):
                emit_slab(s)

            def group_mm(pss, i, s, kdl, first, last):
                for nch in range(NCH):
                    nc.tensor.matmul(
                        pss[nch][:],
                        lhsT=xslabs[s][:, 2 * kdl : 2 * kdl + 2, ts(i, P)],
                        rhs=yslabs[s][:, 2 * kdl : 2 * kdl + 2, ts(nch, FD)],
                        start=first,
                        stop=last,
                        perf_mode=mybir.MatmulPerfMode.DoubleRow,
                    )

            def mm_group(i, kds):
                """One i-tile's accumulation over DR k-steps into 4 banks."""
                pss = [
                    psump.tile([P, FD], f32, name=f"ps{n}", tag="ps")
                    for n in range(NCH)
                ]
                last = len(kds) - 1
                for j, (s, kdl) in enumerate(kds):
                    group_mm(pss, i, s, kdl, j == 0, j == last)
                return pss

            def pair_chase(kds):
                """i0 and i1 march kd-major (8 banks) right behind the DMA
                stream during a chase window; returns their psum banks."""
                pair = [
                    [
                        psump.tile([P, FD], f32, name=f"ps{n}", tag="ps")
                        for n in range(NCH)
                    ]
                    for _ in range(2)
                ]
                last = len(kds) - 1
                for j, (s, kdl) in enumerate(kds):
                    for i in (0, 1):
                        group_mm(pair[i], i, s, kdl, j == 0, j == last)
                return pair

            Q0 = [(0, kdl) for kdl in range(4)]
            Q1 = [(1, kdl) for kdl in range(4)]
            HH = [(2, kdl) for kdl in range(4)] + [(3, kdl) for kdl in range(4)]

            # ---- q0: kd 0..3, spill CAST -> ACC ----
            def chase_mm(ps, i, kdl, nch, last):
                nc.tensor.matmul(
                    ps[:],
                    lhsT=xslabs[0][:, 2 * kdl : 2 * kdl + 2, ts(i, P)],
                    rhs=yslabs[0][:, 2 * kdl : 2 * kdl + 2, ts(nch, FD)],
                    start=(kdl == 0),
                    stop=(kdl == last),
                    perf_mode=mybir.MatmulPerfMode.DoubleRow,
                )

            ps0 = [
                psump.tile([P, FD], f32, name=f"ps{n}", tag="ps")
                for n in range(NCH)
            ]
            ps1 = [
                psump.tile([P, FD], f32, name=f"ps{n}", tag="ps")
                for n in range(3)
            ]
            for kdl in range(4):
                for kol in (2 * kdl, 2 * kdl + 1):
                    warm_mm(xslabs[0], kol)
                    warm_mm(yslabs[0], kol)
                for nch in range(NCH):
                    chase_mm(ps0[nch], 0, kdl, nch, 3)
                for nch in range(3):
                    chase_mm(ps1[nch], 1, kdl, nch, 3)
            for nch in range(NCH):
                nc.vector.tensor_copy(out=ACC[:, 0, nch, :], in_=ps0[nch][:])
            for nch in range(3):
                nc.vector.tensor_copy(out=ACC[:, 1, nch, :], in_=ps1[nch][:])
            ps13 = psump.tile([P, FD], f32, name="ps13", tag="ps")
            for kdl in range(4):
                chase_mm(ps13, 1, kdl, 3, 3)
            nc.vector.tensor_copy(out=ACC[:, 1, 3, :], in_=ps13[:])

            for i in range(2, MT):
                pss = mm_group(i, Q0)
                for nch in range(NCH):
                    nc.vector.tensor_copy(
                        out=ACC[:, i, nch, :], in_=pss[nch][:]
                    )

            # slab 3 (ko 24-31) reuses slab 0's buffers — emit after q0.
            emit_slab(3)

            # ---- q1: kd 4..7, spill ADD -> ACC ----
            for i in range(MT):
                pss = mm_group(i, Q1)
                for nch in range(NCH):
                    nc.vector.tensor_tensor(
                        ACC[:, i, nch, :], pss[nch][:], ACC[:, i, nch, :],
                        mybir.AluOpType.add,
                    )

            # ---- H: kd 8..15, close: PSUM + ACC, scale, out ----
            def close(i, pss):
                for nch in range(NCH):
                    ot = outp.tile([P, FD], f32, name="ot")
                    nc.vector.tensor_tensor(
                        ot[:], pss[nch][:], ACC[:, i, nch, :],
                        mybir.AluOpType.add,
                    )
                    nc.vector.tensor_scalar_mul(ot[:], ot[:], clip_prod[:])
                    nc.sync.dma_start(out[ts(i, P), ts(nch, FD)], ot[:])

            for i in range(MT):
                if i == 0:
                    pss = [
                        psump.tile([P, FD], f32, name=f"ps{n}", tag="ps")
                        for n in range(NCH)
                    ]
                    for j, (s, kdl) in enumerate(HH):
                        for kol in (2 * kdl, 2 * kdl + 1):
                            warm_mm(xslabs[s], kol)
                            warm_mm(yslabs[s], kol)
                        group_mm(pss, i, s, kdl, j == 0, j == len(HH) - 1)
                else:
                    pss = mm_group(i, HH)
                close(i, pss)

    nc.compile()
    _dedupe_ldweights(nc)
    return nc


def _dedupe_ldweights(nc):
    """Drop redundant standalone InstLdweights left by bacc's matmul split.

    Consecutive matmuls sharing one stationary tile still get one
    InstLdweights each; an InstLdweights identical to the previous one
    (same AP, same mode) with no semaphore waits/updates is a no-op."""
    removed = 0
    for blk in nc.m.functions[0].blocks:
        prev_key = None
        keep = []
        for inst in blk.instructions:
            nm = type(inst).__name__
            if nm == "InstLdweights":
                pap = inst.ins[0]
                key = (
                    pap.memref,
                    pap.offset,
                    str(pap.ap),
                    str(pap.dtype),
                    str(inst.perf_mode),
                    str(inst.is_transpose),
                )
                if (
                    key == prev_key
                    and not inst.has_wait()
                    and not inst.has_update()
                ):
                    removed += 1
                    continue
                prev_key = key
            keep.append(inst)
        if removed:
            blk.instructions = keep
    return removed


_PROGRAM_CACHE = None


def _get_program():
    global _PROGRAM_CACHE
    if _PROGRAM_CACHE is None:
        _PROGRAM_CACHE = _build_program()
    return _PROGRAM_CACHE


def _shard_inputs(x, y, x_clip, y_clip):
    x = np.asarray(x, dtype=np.float32)
    y = np.asarray(y, dtype=np.float32)
    clips = np.empty((P, 2), dtype=np.float32)
    clips[:, 0] = np.float32(x_clip)
    clips[:, 1] = np.float32(y_clip)
    in_maps = []
    for c in range(NCORES):
        b, mh, nh = c // 4, (c % 4) // 2, c % 2
        in_maps.append(
            {
                "xsT": np.ascontiguousarray(x[b, mh * MSH : (mh + 1) * MSH, :].T),
                "ysT": np.ascontiguousarray(y[b, nh * NSH : (nh + 1) * NSH, :].T),
                "clips": clips,
            }
        )
    return in_maps


def run_sharded(x, y, x_clip, y_clip, trace=False, **kwargs):
    """Run the SPMD kernel; returns (out, BassKernelResults)."""
    from concourse.bass_utils import run_bass_kernel_spmd

    nc = _get_program()
    in_maps = _shard_inputs(x, y, x_clip, y_clip)
    res = run_bass_kernel_spmd(
        nc, in_maps, core_ids=list(range(NCORES)), trace=trace, **kwargs
    )
    out = np.empty((B, M, N), dtype=np.float32)
    for c in range(NCORES):
        b, mh, nh = c // 4, (c % 4) // 2, c % 2
        out[b, mh * MSH : (mh + 1) * MSH, nh * NSH : (nh + 1) * NSH] = res.results[
            c
        ]["out"]
    return out, res


def kernel(x, y, x_clip, y_clip):
    out, _ = run_sharded(x, y, x_clip, y_clip, trace=False)
    return out
